# revision 46
# baseline (speedup 1.0000x reference)
import os
import sys

for _p in ("/opt/trn_rl_repo", "/root/.axon_site/_ro/trn_rl_repo"):
    if os.path.isdir(_p) and _p not in sys.path:
        sys.path.insert(0, _p)

from contextlib import ExitStack

import numpy as np

import concourse.bass as bass
import concourse.tile as tile
from concourse import bacc, mybir
from concourse.bass_utils import run_bass_kernel_spmd
from concourse.masks import make_identity

# Problem shapes (hardcoded per spec): cross-attention
#   q = input1 @ W^T + b ; attn = softmax(q @ input2^T) ;
#   o1 = attn @ input2 ; o2 = attn^T @ input1
B, N1, N2, D = 8, 2048, 2048, 512

PT = 128            # partition tile
NT = N1 // PT       # 16 query row-tiles
MT = N2 // PT       # 16 key row-tiles
KT = D // PT        # 4 contraction tiles over D
CHUNK = 512         # moving-dim chunk (PSUM bank = 512 fp32)
MC = N2 // CHUNK    # 4 chunks of keys

F32 = mybir.dt.float32
F32R = mybir.dt.float32r
MM_DT = mybir.dt.float32r  # S-path matmul dtype: float32 (exact) or float32r (fast)
BF16 = mybir.dt.bfloat16  # storage dtype for P / B / A_scaled (range!)
FP16 = mybir.dt.float16   # S-path operand dtype: fp16 = 8x bf16 mantissa
AF = mybir.ActivationFunctionType
AX = mybir.AxisListType
ALU = mybir.AluOpType

SHIFT = 100.0  # constant softmax stabilizer: S~N(0,22.6^2) so exp(S-SHIFT)
               # can't overflow (needs S>188 = 8.3 sigma) and a row's top
               # term can't underflow (needs rowmax<12 = 0.5 sigma over 2048)


def _build(nreps=1):
    """One NeuronCore program: full cross-attention for ONE batch sample.

    Math (A = input1[b] [N1,D], Bm = input2[b] [N2,D], W [D,D], bvec [D]):
      C^T[d,m]   = sum_o W[o,d] * Bm[m,o]          (projected keys, W natural as lhsT)
      bias_row[m]= sum_o bvec[o] * Bm[m,o]
      S[n,m]     = sum_d A[n,d] * C[m,d] + bias_row[m]   (bias folded as exp factor)
      P[n,m]     = exp(S - SHIFT) * E[m]  (bf16), rowsum via DVE accum
      o1[n,d]    = (1/rowsum[n]) * sum_m P^T[m,n] * Bm[m,d]  (P^T via XBAR DMA)
      o2[m,d]    = sum_n P[n,m] * (A[n,d]/rowsum[n])
    """
    nc = bacc.Bacc("TRN2", target_bir_lowering=False, debug=False, num_devices=B)
    a_d = nc.dram_tensor("a", [N1, D], MM_DT, kind="ExternalInput").ap()
    b_d = nc.dram_tensor("bm", [N2, D], MM_DT, kind="ExternalInput").ap()
    w_d = nc.dram_tensor("w", [D, D], MM_DT, kind="ExternalInput").ap()
    bv_d = nc.dram_tensor("bvec", [D], MM_DT, kind="ExternalInput").ap()
    ones_d = nc.dram_tensor("ones", [1, PT], MM_DT, kind="ExternalInput").ap()
    eye_d = nc.dram_tensor("eye", [PT, PT], MM_DT, kind="ExternalInput").ap()
    eye16_d = nc.dram_tensor("eye16", [PT, PT], FP16, kind="ExternalInput").ap()
    o1_d = nc.dram_tensor("o1", [N1, D], F32, kind="ExternalOutput").ap()
    o2_d = nc.dram_tensor("o2", [N2, D], F32, kind="ExternalOutput").ap()

    # PSUM can only be read by DVE and ACT (GPSIMD/Pool is SBUF-only), so
    # PSUM-drain copies alternate between those two engines.
    def cp2(i, out, in_):
        if i % 2 == 0:
            nc.vector.tensor_copy(out, in_)
        else:
            nc.scalar.copy(out, in_)

    def cp31(i, out, in_):
        # 3:1 DVE:ACT (ACT also carries the exp in phase 1)
        if i % 4 == 3:
            nc.scalar.copy(out, in_)
        else:
            nc.vector.tensor_copy(out, in_)

    with tile.TileContext(nc) as tc:
      for rep in range(nreps):
        sfx = f"r{rep}"
        big = ExitStack()
        const = big.enter_context(tc.tile_pool(name=f"const{sfx}", bufs=1))
        ident = const.tile([PT, PT], MM_DT, name="ident", tag="ident")
        nc.scalar.dma_start(ident[:], eye_d[:])
        ident16 = const.tile([PT, PT], FP16, name="ident16", tag="ident16")
        nc.scalar.dma_start(ident16[:], eye16_d[:])
        ones_row = const.tile([1, PT], MM_DT, name="ones", tag="ones")
        nc.scalar.dma_start(ones_row[:], ones_d[:])
        ebc = const.tile([PT, N2], BF16, name="ebc", tag="ebc")
        nshift = const.tile([PT, 1], F32, name="nshift", tag="nshift")
        nc.vector.memset(nshift[:], -SHIFT)

        stats = big.enter_context(tc.tile_pool(name=f"stats{sfx}", bufs=1))
        recip_all = stats.tile([PT, NT], F32, name="recip", tag="recip")
        # bias path: bias_col[m%128, mt] = sum_o B[m,o] b[o] accumulated on
        # DVE (free-axis reduce), exp'd, PE-transposed, row-assembled by a
        # small DMA, then partition-broadcast to ebc -- no PE bias matmuls.
        bias_col = stats.tile([PT, MT], F32, name="biascol", tag="biascol")
        ecol = stats.tile([PT, MT], FP16, name="ecol", tag="ecol")
        bvb = stats.tile([PT, D], FP16, name="bvb", tag="bvb")
        bvf = stats.tile([1, D], MM_DT, name="bvf", tag="bvf")
        bv16 = stats.tile([1, D], FP16, name="bv16", tag="bv16")
        erow_t = stats.tile([PT, PT], BF16, name="erowt", tag="erowt")
        erow1 = stats.tile([1, N2], BF16, name="erow1", tag="erow1")
        nc.scalar.dma_start(bvf[0:1, :], bv_d.rearrange("(a d) -> a d", a=1))
        nc.gpsimd.tensor_copy(bv16[0:1, :], bvf[0:1, :])
        nc.gpsimd.partition_broadcast(bvb[:], bv16[0:1, :])

        # P^T XBAR landing tiles + guard scratch: long-lived pool created
        # before the phase-0 scoped pools (no byte-sharing with them).
        ptp = big.enter_context(tc.tile_pool(name=f"ptp{sfx}", bufs=1))
        gscr = ptp.tile([1, N2], BF16, name="gscr", tag="gscr")
        # A^T stored as one wide tile: column block k holds the k-th 128-row
        # slice of the transpose ([d-part, k*N1 + col]).
        at_pool = big.enter_context(tc.tile_pool(name=f"atp{sfx}", bufs=1))
        at_all = at_pool.tile([PT, KT * N1], FP16, name="at", tag="at")
        ct_pool = big.enter_context(tc.tile_pool(name=f"ctp{sfx}", bufs=1))
        CT = [ct_pool.tile([PT, N2], FP16, name=f"ct{k}", tag=f"ct{k}") for k in range(KT)]
        bbf_pool = big.enter_context(tc.tile_pool(name=f"bbfp{sfx}", bufs=1))
        Bbf = [bbf_pool.tile([PT, D], BF16, name=f"bbf{t}", tag=f"bbf{t}") for t in range(MT)]

        # ---------------- phase 0: load + transposes + projection ----------
        # Asc tiles double as the bf16 stash of raw A (scaled in place once
        # the rowsum reciprocal is known) so A is only loaded once.
        asc_pool = big.enter_context(tc.tile_pool(name=f"ascp{sfx}", bufs=1))
        Asc = [asc_pool.tile([PT, D], BF16, name=f"asc{t}", tag=f"asc{t}") for t in range(NT)]
        st16_pool = big.enter_context(tc.tile_pool(name=f"st16p{sfx}", bufs=1))
        B16 = [st16_pool.tile([PT, D], FP16, name=f"b16_{t}", tag=f"b16_{t}") for t in range(MT)]
        A16 = [st16_pool.tile([PT, D], FP16, name=f"a16_{t}", tag=f"a16_{t}") for t in range(NT)]
        with ExitStack() as ph0:
            wp = ph0.enter_context(tc.tile_pool(name=f"wp{sfx}", bufs=1))
            Wb = wp.tile([PT, KT * D], FP16, name="wb", tag="wb")
            btp = ph0.enter_context(tc.tile_pool(name=f"btp{sfx}", bufs=1))
            bt_all = btp.tile([PT, KT * N2], FP16, name="bt", tag="bt")
            ldp = ph0.enter_context(tc.tile_pool(name=f"ldp{sfx}", bufs=1))
            ps0 = ph0.enter_context(tc.tile_pool(name=f"ps0{sfx}", bufs=1, space="PSUM"))

            # 4 transposes of a load tile share one PSUM bank; one strided
            # copy drains them into the column-t slices of the 4 k-blocks.
            def tr_tile(i, src, dst_all, dt=FP16, idn=None):
                trp4 = ps0.tile([PT, D], dt, name="tr", tag="tr", bufs=2)
                for k in range(KT):
                    nc.tensor.transpose(
                        trp4[:, k * PT:(k + 1) * PT],
                        src[:, k * PT:(k + 1) * PT],
                        (idn if idn is not None else ident16)[:])
                dst = dst_all[:].rearrange("p (k n) -> p k n", k=KT)[
                    :, :, i * PT:(i + 1) * PT]
                cp2(i, dst, trp4[:].rearrange("p (k n) -> p k n", k=KT))

            def load_group(src_d, g, n_t, split=1):
                # one DMA (or `split` DMAs) covering n_t row-tiles
                grp = ldp.tile([PT, n_t * D], MM_DT, name="ld", tag="ld", bufs=4)
                step = n_t // split
                for s in range(split):
                    t0 = g * n_t + s * step
                    nc.sync.dma_start(
                        grp[:, s * step * D:(s + 1) * step * D].rearrange(
                            "p (t d) -> p t d", t=step),
                        src_d[t0 * PT:(t0 + step) * PT, :].rearrange(
                            "(t p) d -> p t d", p=PT))
                return grp

            def proj_chunk(mc):
                # C^T[d2, m-chunk] = sum_o W[o, d2] * B^T[o, m-chunk]
                cps = [ps0.tile([PT, CHUNK], F32, name="mm", tag="mm", bufs=5)
                       for _ in range(KT)]
                for ko in range(KT):
                    for k2 in range(KT):
                        nc.tensor.matmul(
                            cps[k2][:],
                            Wb[:, ko * D + k2 * PT:ko * D + (k2 + 1) * PT],
                            bt_all[:, ko * N2 + mc * CHUNK:ko * N2 + (mc + 1) * CHUNK],
                            start=(ko == 0), stop=(ko == KT - 1),
                        )
                for k2 in range(KT):
                    cp2(k2, CT[k2][:, mc * CHUNK:(mc + 1) * CHUNK], cps[k2][:])

            # Load order keeps the DMA wire busy with what the PE needs
            # next: B group 0 (split for latency), B1, W (first projection
            # consumer), then A groups interleaved with remaining B groups.
            bgrps = {0: load_group(b_d, 0, 4, split=4)}
            # gpsimd SWDGE DMA casts f32 -> fp16 in flight (separate queue)
            nc.gpsimd.dma_start(Wb[:].rearrange("p (k d) -> p k d", k=KT),
                                w_d.rearrange("(k p) d -> p k d", p=PT))
            bgrps[1] = load_group(b_d, 1, 4)
            agrps = {}
            for g in range(2, 4):
                agrps[g - 2] = load_group(a_d, g - 2, 4)
                bgrps[g] = load_group(b_d, g, 4)
            for g in range(2, 4):
                agrps[g] = load_group(a_d, g, 4)

            # B: transpose each 4-tile group then project the chunk it feeds.
            bjunk = ldp.tile([PT, D], FP16, name="bjunk", tag="bjunk", bufs=2)
            for mc in range(MC):
                grp = bgrps[mc]
                for q in range(4):
                    t = 4 * mc + q
                    nc.gpsimd.tensor_copy(Bbf[t][:], grp[:, q * D:(q + 1) * D])
                    if t % 2 == 0:
                        nc.vector.tensor_copy(B16[t][:], grp[:, q * D:(q + 1) * D])
                    else:
                        nc.scalar.copy(B16[t][:], grp[:, q * D:(q + 1) * D])
                    if mc == 0:
                        tr_tile(t, grp[:, q * D:(q + 1) * D], bt_all,
                                dt=MM_DT, idn=ident)
                    else:
                        tr_tile(t, B16[t], bt_all)
                    nc.vector.scalar_tensor_tensor(
                        bjunk[:], B16[t][:], 1.0, bvb[:],
                        op0=ALU.mult, op1=ALU.mult,
                        accum_out=bias_col[:, t:t + 1])
                proj_chunk(mc)
            # E = exp(bias); transpose [m%128, mt] -> [mt, m%128], assemble a
            # single row, and broadcast across partitions for the P multiply.
            nc.scalar.activation(ecol[:], bias_col[:], AF.Exp)
            ept = ps0.tile([PT, PT], FP16, name="ept", tag="ept", bufs=1)
            nc.tensor.transpose(ept[0:MT, :], ecol[:, 0:MT], ident16[:])
            nc.vector.tensor_copy(erow_t[0:MT, :], ept[0:MT, :])
            nc.scalar.dma_start(
                erow1[0:1, :].rearrange("a (t l) -> a t l", t=MT),
                erow_t[0:MT, :])
            nc.gpsimd.partition_broadcast(ebc[:], erow1[0:1, :])
            # A: bf16 stash for o2 (scaled in place later) + fp16 for S.
            for g in range(MC):
                grp = agrps[g]
                for q in range(4):
                    t = 4 * g + q
                    nc.gpsimd.tensor_copy(Asc[t][:], grp[:, q * D:(q + 1) * D])
                    if t % 2 == 0:
                        nc.vector.tensor_copy(A16[t][:], grp[:, q * D:(q + 1) * D])
                    else:
                        nc.scalar.copy(A16[t][:], grp[:, q * D:(q + 1) * D])
                    tr_tile(t, A16[t], at_all)

        # ------ phase 1: per query-tile S -> P -> P^T (XBAR) -> o1 ---------
        # Software-pipelined: A^T transposes (4-tile groups) lead the S
        # stream by one group; o1(nt) trails S(nt) by O1_LAG so the exp/
        # rowsum/XBAR-transpose latency hides under the next S tiles.
        O1_LAG = 3
        p_pool = big.enter_context(tc.tile_pool(name=f"pp{sfx}", bufs=1))
        Pt = [p_pool.tile([PT, N2], BF16, name=f"p{t}", tag=f"p{t}") for t in range(NT)]
        with ExitStack() as ph12:
            ps1 = ph12.enter_context(tc.tile_pool(name=f"ps1{sfx}", bufs=1, space="PSUM"))
            smp = ph12.enter_context(tc.tile_pool(name=f"smp{sfx}", bufs=1))
            outp = ph12.enter_context(tc.tile_pool(name=f"outp{sfx}", bufs=1))

            def stage_s(nt):
                spsums = [ps1.tile([PT, CHUNK], F32, name="s", tag="s", bufs=5)
                          for _ in range(MC)]
                # k-outer: stationary A^T slice shared by 4 consecutive MMs
                for k in range(KT):
                    for mc in range(MC):
                        nc.tensor.matmul(
                            spsums[mc][:],
                            at_all[:, k * N1 + nt * PT:k * N1 + (nt + 1) * PT],
                            CT[k][:, mc * CHUNK:(mc + 1) * CHUNK],
                            start=(k == 0), stop=(k == KT - 1),
                        )
                sums = []
                for mc in range(MC):
                    tch = smp.tile([PT, CHUNK], BF16, name="texp", tag="texp", bufs=4)
                    nc.scalar.activation(
                        tch[:], spsums[mc][:], AF.Exp,
                        bias=nshift[:], scale=1.0,
                    )
                    sm = smp.tile([PT, 1], F32, name="sum", tag="sum", bufs=8)
                    # P = T * E[m] (bias fold); accum gives the weighted rowsum
                    nc.vector.scalar_tensor_tensor(
                        Pt[nt][:, mc * CHUNK:(mc + 1) * CHUNK],
                        tch[:], 1.0,
                        ebc[:, mc * CHUNK:(mc + 1) * CHUNK],
                        op0=ALU.mult, op1=ALU.mult, accum_out=sm[:],
                    )
                    sums.append(sm)
                s01 = smp.tile([PT, 1], F32, name="s01", tag="s01", bufs=2)
                nc.vector.tensor_add(s01[:], sums[0][:], sums[1][:])
                s23 = smp.tile([PT, 1], F32, name="s23", tag="s23", bufs=2)
                nc.vector.tensor_add(s23[:], sums[2][:], sums[3][:])
                stot = smp.tile([PT, 1], F32, name="stot", tag="stot", bufs=2)
                nc.vector.tensor_add(stot[:], s01[:], s23[:])
                nc.vector.reciprocal(recip_all[:, nt:nt + 1], stot[:])
                # scale the stashed bf16 A tile in place by 1/rowsum
                nc.gpsimd.tensor_scalar_mul(Asc[nt][:], Asc[nt][:], recip_all[:, nt:nt + 1])
                # Guard: a tiny regular DMA reading one row of Pt[nt] on the
                # same HWDGE queue as the XBAR below. Its (reliable) sem
                # waits cover all four stt chunk writes, and in-order queue
                # dispatch then guarantees the XBAR reads finished data --
                # the XBAR ucode's own input waits proved untrustworthy.
                nc.sync.dma_start(gscr[0:1, :], Pt[nt][0:1, :])
                ptall = ptp.tile([PT, N2], BF16, name="ptx", tag="ptx", bufs=O1_LAG + 1)
                nc.sync.dma_start_transpose(
                    ptall[:].rearrange("p (b l) -> p b l", b=MT), Pt[nt][:])
                return ptall

            def stage_o1(nt, ptall):
                o1ps = ps1.tile([PT, D], F32, name="o1", tag="o1", bufs=1)
                for mt in range(MT):
                    nc.tensor.matmul(o1ps[:], ptall[:, mt * PT:(mt + 1) * PT],
                                     Bbf[mt][:],
                                     start=(mt == 0), stop=(mt == MT - 1))
                o1sb = outp.tile([PT, D], F32, name="o1sb", tag="o1sb", bufs=2)
                nc.scalar.mul(o1sb[:], o1ps[:], recip_all[:, nt:nt + 1])
                nc.sync.dma_start(o1_d[nt * PT:(nt + 1) * PT, :], o1sb[:])

            H = D // 2

            def stage_o2(mt):
                o2ps = ps1.tile([PT, D], F32, name="o2", tag="o2", bufs=2)
                o2sb = outp.tile([PT, D], F32, name="o2sb", tag="o2sb", bufs=2)
                for h in range(2):
                    for nt in range(NT):
                        nc.tensor.matmul(
                            o2ps[:, h * H:(h + 1) * H],
                            Pt[nt][:, mt * PT:(mt + 1) * PT],
                            Asc[nt][:, h * H:(h + 1) * H],
                            start=(nt == 0), stop=(nt == NT - 1))
                    cp2(mt + h, o2sb[:, h * H:(h + 1) * H], o2ps[:, h * H:(h + 1) * H])
                    if mt >= MT - 2:
                        nc.sync.dma_start(
                            o2_d[mt * PT:(mt + 1) * PT, h * H:(h + 1) * H],
                            o2sb[:, h * H:(h + 1) * H])
                if mt < MT - 2:
                    nc.sync.dma_start(o2_d[mt * PT:(mt + 1) * PT, :], o2sb[:])

            pts = {}
            for nt in range(NT):
                pts[nt] = stage_s(nt)
                if nt - O1_LAG >= 0:
                    stage_o1(nt - O1_LAG, pts.pop(nt - O1_LAG))
            # epilogue: pair each remaining o1 with an o2 tile so the PE has
            # ready work while the last XBAR transposes land
            mt_next = 0
            for nt in range(NT - O1_LAG, NT):
                stage_o2(mt_next); mt_next += 1
                stage_o1(nt, pts.pop(nt))
            for mt in range(mt_next, MT):
                stage_o2(mt)
        big.close()
    nc.compile()
    return nc


_state = {}


def _get_nc(nreps=1):
    key = f"nc{nreps}"
    if key not in _state:
        _state[key] = _build(nreps)
    return _state[key]


def _in_maps(input1, input2, W_w, W_b):
    return [
        {
            "a": np.ascontiguousarray(input1[bb], dtype=np.float32),
            "bm": np.ascontiguousarray(input2[bb], dtype=np.float32),
            "w": np.ascontiguousarray(W_w, dtype=np.float32),
            "bvec": np.ascontiguousarray(W_b, dtype=np.float32),
            "ones": np.ones((1, PT), dtype=np.float32),
            "eye": np.eye(PT, dtype=np.float32),
            "eye16": np.eye(PT, dtype=np.float16),
        }
        for bb in range(B)
    ]


def kernel(input1, input2, W_w, W_b):
    res = run_bass_kernel_spmd(
        _get_nc(), _in_maps(input1, input2, W_w, W_b), core_ids=list(range(B))
    )
    o1 = np.stack([r["o1"] for r in res.results])
    o2 = np.stack([r["o2"] for r in res.results])
    return o1, o2


def _pjrt_fn(nc, in_maps, donate=False):
    """Build a single-call jitted runner for `nc` (copy of run_bass_via_pjrt
    multi-core path, without donation so device inputs can be reused)."""
    import jax
    import numpy as np_
    from jax.sharding import Mesh, NamedSharding, PartitionSpec
    from jax.experimental.shard_map import shard_map

    from concourse import mybir as _mybir
    from concourse.bass2jax import (
        _bass_exec_p,
        install_neuronx_cc_hook,
        partition_id_tensor,
    )

    install_neuronx_cc_hook()
    partition_name = nc.partition_id_tensor.name if nc.partition_id_tensor else None

    in_names, out_names, out_avals, zero_outs = [], [], [], []
    for alloc in nc.m.functions[0].allocations:
        if not isinstance(alloc, _mybir.MemoryLocationSet):
            continue
        name = alloc.memorylocations[0].name
        if alloc.kind == "ExternalInput":
            if name != partition_name:
                in_names.append(name)
        elif alloc.kind == "ExternalOutput":
            out_names.append(name)
            shape = tuple(alloc.tensor_shape)
            dtype = _mybir.dt.np(alloc.dtype)
            out_avals.append(jax.core.ShapedArray(shape, dtype))
            zero_outs.append(np_.zeros(shape, dtype))

    all_in = list(in_names) + list(out_names)
    if partition_name is not None:
        all_in.append(partition_name)

    def _body(*args):
        operands = list(args)
        if partition_name is not None:
            operands.append(partition_id_tensor())
        outs = _bass_exec_p.bind(
            *operands,
            out_avals=tuple(out_avals),
            in_names=tuple(all_in),
            out_names=tuple(out_names),
            lowering_input_output_aliases=(),
            sim_require_finite=True,
            sim_require_nnan=True,
            nc=nc,
        )
        return tuple(outs)

    devices = jax.devices()[:B]
    mesh = Mesh(np_.asarray(devices), ("core",))
    nargs = len(in_names) + len(out_names)
    sh = NamedSharding(mesh, PartitionSpec("core"))
    fn = jax.jit(
        shard_map(
            _body, mesh=mesh,
            in_specs=(PartitionSpec("core"),) * nargs,
            out_specs=(PartitionSpec("core"),) * len(out_names),
            check_rep=False,
        ),
        **({"donate_argnums": tuple(range(len(in_names), nargs))} if donate else {}),
    )
    args = [
        jax.device_put(np_.concatenate([m[n] for m in in_maps], axis=0), sh)
        for n in in_names
    ] + [
        jax.device_put(np_.concatenate([z] * B, axis=0), sh) for z in zero_outs
    ]
    return fn, args, out_names, out_avals


def _time_fn(fn, args, calls=30, reps=4):
    """Pipelined timing: issue `calls` executions, block once at the end.
    Returns list of per-call ns (one value per rep)."""
    import time

    import jax

    r = fn(*args)
    jax.block_until_ready(r)
    out = []
    for _ in range(reps):
        t0 = time.perf_counter()
        for _ in range(calls):
            r = fn(*args)
        jax.block_until_ready(r)
        out.append((time.perf_counter() - t0) / calls * 1e9)
    return out


def bench_hw(input1, input2, W_w, W_b, calls=40):
    """HW body time via 2-rep minus 1-rep NEFF wall times (dispatch cancels).
    Returns (body_ns, t1_list_p, t2_list_p)."""
    in_maps = _in_maps(input1, input2, W_w, W_b)
    fn1, args1, _, _ = _pjrt_fn(_get_nc(1), in_maps)
    fn2, args2, _, _ = _pjrt_fn(_get_nc(2), in_maps)
    t1 = _time_fn(fn1, args1, calls)
    t2 = _time_fn(fn2, args2, calls)
    import numpy as np_
    p = lambda ts, q: float(np_.percentile(ts, q))
    body = p(t2, 10) - p(t1, 10)
    return body, (p(t1,10), p(t1,50)), (p(t2,10), p(t2,50))


# revision 71
# speedup vs baseline: 1.1298x; 1.1298x over previous
import os
import sys

for _p in ("/opt/trn_rl_repo", "/root/.axon_site/_ro/trn_rl_repo"):
    if os.path.isdir(_p) and _p not in sys.path:
        sys.path.insert(0, _p)

from contextlib import ExitStack

import numpy as np

import concourse.bass as bass
import concourse.tile as tile
from concourse import bacc, mybir
from concourse.bass_utils import run_bass_kernel_spmd
from concourse.masks import make_identity

# Problem shapes (hardcoded per spec): cross-attention
#   q = input1 @ W^T + b ; attn = softmax(q @ input2^T) ;
#   o1 = attn @ input2 ; o2 = attn^T @ input1
B, N1, N2, D = 8, 2048, 2048, 512

PT = 128            # partition tile
NT = N1 // PT       # 16 query row-tiles
MT = N2 // PT       # 16 key row-tiles
KT = D // PT        # 4 contraction tiles over D
CHUNK = 512         # moving-dim chunk (PSUM bank = 512 fp32)
MC = N2 // CHUNK    # 4 chunks of keys

F32 = mybir.dt.float32
F32R = mybir.dt.float32r
MM_DT = mybir.dt.float32r  # S-path matmul dtype: float32 (exact) or float32r (fast)
BF16 = mybir.dt.bfloat16  # storage dtype for P / B / A_scaled (range!)
FP16 = mybir.dt.float16   # S-path operand dtype: fp16 = 8x bf16 mantissa
AF = mybir.ActivationFunctionType
AX = mybir.AxisListType
ALU = mybir.AluOpType

SHIFT = 100.0  # constant softmax stabilizer: S~N(0,22.6^2) so exp(S-SHIFT)
               # can't overflow (needs S>188 = 8.3 sigma) and a row's top
               # term can't underflow (needs rowmax<12 = 0.5 sigma over 2048)


def _build(nreps=1):
    """One NeuronCore program: full cross-attention for ONE batch sample.

    Math (A = input1[b] [N1,D], Bm = input2[b] [N2,D], W [D,D], bvec [D]):
      C^T[d,m]   = sum_o W[o,d] * Bm[m,o]          (projected keys, W natural as lhsT)
      bias_row[m]= sum_o bvec[o] * Bm[m,o]
      S[n,m]     = sum_d A[n,d] * C[m,d] + bias_row[m]   (bias folded as exp factor)
      P[n,m]     = exp(S - SHIFT) * E[m]  (bf16), rowsum via DVE accum
      o1[n,d]    = (1/rowsum[n]) * sum_m P^T[m,n] * Bm[m,d]  (P^T via XBAR DMA)
      o2[m,d]    = sum_n P[n,m] * (A[n,d]/rowsum[n])
    """
    nc = bacc.Bacc("TRN2", target_bir_lowering=False, debug=False, num_devices=B)
    a_d = nc.dram_tensor("a", [N1, D], MM_DT, kind="ExternalInput").ap()
    b_d = nc.dram_tensor("bm", [N2, D], MM_DT, kind="ExternalInput").ap()
    w_d = nc.dram_tensor("w", [D, D], MM_DT, kind="ExternalInput").ap()
    bv_d = nc.dram_tensor("bvec", [D], MM_DT, kind="ExternalInput").ap()
    ones_d = nc.dram_tensor("ones", [1, PT], MM_DT, kind="ExternalInput").ap()
    eye_d = nc.dram_tensor("eye", [PT, PT], MM_DT, kind="ExternalInput").ap()
    eye16_d = nc.dram_tensor("eye16", [PT, PT], FP16, kind="ExternalInput").ap()
    o1_d = nc.dram_tensor("o1", [N1, D], F32, kind="ExternalOutput").ap()
    o2_d = nc.dram_tensor("o2", [N2, D], F32, kind="ExternalOutput").ap()

    # PSUM can only be read by DVE and ACT (GPSIMD/Pool is SBUF-only), so
    # PSUM-drain copies alternate between those two engines.
    def cp2(i, out, in_):
        if i % 2 == 0:
            nc.vector.tensor_copy(out, in_)
        else:
            nc.scalar.copy(out, in_)

    def cp31(i, out, in_):
        # 3:1 DVE:ACT (ACT also carries the exp in phase 1)
        if i % 4 == 3:
            nc.scalar.copy(out, in_)
        else:
            nc.vector.tensor_copy(out, in_)

    with tile.TileContext(nc) as tc:
      for rep in range(nreps):
        sfx = f"r{rep}"
        big = ExitStack()
        const = big.enter_context(tc.tile_pool(name=f"const{sfx}", bufs=1))
        ident = const.tile([PT, PT], MM_DT, name="ident", tag="ident")
        nc.scalar.dma_start(ident[:], eye_d[:])
        ident16 = const.tile([PT, PT], FP16, name="ident16", tag="ident16")
        make_identity(nc, ident16[:])
        ones_row = const.tile([1, PT], MM_DT, name="ones", tag="ones")
        nc.scalar.dma_start(ones_row[:], ones_d[:])
        ebc = const.tile([PT, N2], BF16, name="ebc", tag="ebc")
        nshift = const.tile([PT, 1], F32, name="nshift", tag="nshift")
        nc.vector.memset(nshift[:], -SHIFT)

        stats = big.enter_context(tc.tile_pool(name=f"stats{sfx}", bufs=1))
        recip_all = stats.tile([PT, NT], F32, name="recip", tag="recip")
        # bias path: bias_col[m%128, mt] = sum_o B[m,o] b[o] accumulated on
        # DVE (free-axis reduce), exp'd, PE-transposed, row-assembled by a
        # small DMA, then partition-broadcast to ebc -- no PE bias matmuls.
        bias_col = stats.tile([PT, MT], F32, name="biascol", tag="biascol")
        ecol = stats.tile([PT, MT], FP16, name="ecol", tag="ecol")
        bvb = stats.tile([PT, D], FP16, name="bvb", tag="bvb")
        bvf = stats.tile([1, D], MM_DT, name="bvf", tag="bvf")
        bv16 = stats.tile([1, D], FP16, name="bv16", tag="bv16")
        erow_t = stats.tile([PT, PT], BF16, name="erowt", tag="erowt")
        erow1 = stats.tile([1, N2], BF16, name="erow1", tag="erow1")
        nc.scalar.dma_start(bvf[0:1, :], bv_d.rearrange("(a d) -> a d", a=1))
        nc.gpsimd.tensor_copy(bv16[0:1, :], bvf[0:1, :])
        nc.gpsimd.partition_broadcast(bvb[:], bv16[0:1, :])

        # P^T XBAR landing tiles + guard scratch: long-lived pool created
        # before the phase-0 scoped pools (no byte-sharing with them).
        ptp = big.enter_context(tc.tile_pool(name=f"ptp{sfx}", bufs=1))
        gscr = ptp.tile([1, N2], BF16, name="gscr", tag="gscr")
        # A^T stored as one wide tile: column block k holds the k-th 128-row
        # slice of the transpose ([d-part, k*N1 + col]).
        at_pool = big.enter_context(tc.tile_pool(name=f"atp{sfx}", bufs=1))
        at_all = at_pool.tile([PT, KT * N1], FP16, name="at", tag="at")
        ct_pool = big.enter_context(tc.tile_pool(name=f"ctp{sfx}", bufs=1))
        CT = [ct_pool.tile([PT, N2], FP16, name=f"ct{k}", tag=f"ct{k}") for k in range(KT)]
        bbf_pool = big.enter_context(tc.tile_pool(name=f"bbfp{sfx}", bufs=1))
        Bbf = [bbf_pool.tile([PT, D], BF16, name=f"bbf{t}", tag=f"bbf{t}") for t in range(MT)]

        # ---------------- phase 0: load + transposes + projection ----------
        # Asc tiles double as the bf16 stash of raw A (scaled in place once
        # the rowsum reciprocal is known) so A is only loaded once.
        asc_pool = big.enter_context(tc.tile_pool(name=f"ascp{sfx}", bufs=1))
        Asc = [asc_pool.tile([PT, D], BF16, name=f"asc{t}", tag=f"asc{t}") for t in range(NT)]
        st16_pool = big.enter_context(tc.tile_pool(name=f"st16p{sfx}", bufs=1))
        B16 = [st16_pool.tile([PT, D], FP16, name=f"b16_{t}", tag=f"b16_{t}") for t in range(MT)]
        A16 = [st16_pool.tile([PT, D], FP16, name=f"a16_{t}", tag=f"a16_{t}") for t in range(NT)]
        with ExitStack() as ph0:
            wp = ph0.enter_context(tc.tile_pool(name=f"wp{sfx}", bufs=1))
            Wb = wp.tile([PT, KT * D], FP16, name="wb", tag="wb")
            btp = ph0.enter_context(tc.tile_pool(name=f"btp{sfx}", bufs=1))
            bt_all = btp.tile([PT, KT * N2], FP16, name="bt", tag="bt")
            ldp = ph0.enter_context(tc.tile_pool(name=f"ldp{sfx}", bufs=1))
            ps0 = ph0.enter_context(tc.tile_pool(name=f"ps0{sfx}", bufs=1, space="PSUM"))

            # 4 transposes of a load tile share one PSUM bank; one strided
            # copy drains them into the column-t slices of the 4 k-blocks.
            def tr_tile(i, src, dst_all, dt=FP16, idn=None, drain=None):
                trp4 = ps0.tile([PT, D], dt, name="tr", tag="tr", bufs=3)
                for k in range(KT):
                    nc.tensor.transpose(
                        trp4[:, k * PT:(k + 1) * PT],
                        src[:, k * PT:(k + 1) * PT],
                        (idn if idn is not None else ident16)[:])
                dst = dst_all[:].rearrange("p (k n) -> p k n", k=KT)[
                    :, :, i * PT:(i + 1) * PT]
                (drain or nc.vector.tensor_copy)(
                    dst, trp4[:].rearrange("p (k n) -> p k n", k=KT))

            def load_group(src_d, g, n_t, split=1):
                # one DMA (or `split` DMAs) covering n_t row-tiles
                grp = ldp.tile([PT, n_t * D], MM_DT, name="ld", tag="ld", bufs=4)
                step = n_t // split
                for s in range(split):
                    t0 = g * n_t + s * step
                    nc.sync.dma_start(
                        grp[:, s * step * D:(s + 1) * step * D].rearrange(
                            "p (t d) -> p t d", t=step),
                        src_d[t0 * PT:(t0 + step) * PT, :].rearrange(
                            "(t p) d -> p t d", p=PT))
                return grp

            def proj_chunk(mc):
                # C^T[d2, m-chunk] = sum_o W[o, d2] * B^T[o, m-chunk]
                cps = [ps0.tile([PT, CHUNK], F32, name="mm", tag="mm", bufs=4)
                       for _ in range(KT)]
                for k2 in range(KT):
                    for ko in range(KT):
                        nc.tensor.matmul(
                            cps[k2][:],
                            Wb[:, ko * D + k2 * PT:ko * D + (k2 + 1) * PT],
                            bt_all[:, ko * N2 + mc * CHUNK:ko * N2 + (mc + 1) * CHUNK],
                            start=(ko == 0), stop=(ko == KT - 1),
                        )
                for k2 in range(KT):
                    cp2(k2, CT[k2][:, mc * CHUNK:(mc + 1) * CHUNK], cps[k2][:])

            # Load order keeps the DMA wire busy with what the PE needs
            # next: B group 0 (split for latency), B1, W (first projection
            # consumer), then A groups interleaved with remaining B groups.
            bgrps = {0: load_group(b_d, 0, 4, split=4)}
            bgrps[1] = load_group(b_d, 1, 4)
            agrps = {0: load_group(a_d, 0, 4)}
            # gpsimd SWDGE DMA casts f32 -> fp16 in flight (separate queue);
            # emitted late so its wire slot doesn't delay ident/b1-b3
            nc.gpsimd.dma_start(Wb[:].rearrange("p (k d) -> p k d", k=KT),
                                w_d.rearrange("(k p) d -> p k d", p=PT))
            bgrps[2] = load_group(b_d, 2, 4)
            agrps[1] = load_group(a_d, 1, 4)
            bgrps[3] = load_group(b_d, 3, 4)
            for g in range(2, 4):
                agrps[g] = load_group(a_d, g, 4)

            # B: transpose each 4-tile group then project the chunk it feeds.
            bjunk = ldp.tile([PT, D], FP16, name="bjunk", tag="bjunk", bufs=2)
            for mc in range(MC):
                grp = bgrps[mc]
                for q in range(4):
                    t = 4 * mc + q
                    nc.gpsimd.tensor_copy(Bbf[t][:], grp[:, q * D:(q + 1) * D])
                    nc.vector.tensor_copy(B16[t][:], grp[:, q * D:(q + 1) * D])
                    if mc == 0:
                        tr_tile(t, grp[:, q * D:(q + 1) * D], bt_all,
                                dt=MM_DT, idn=ident, drain=nc.scalar.copy)
                    else:
                        tr_tile(t, B16[t], bt_all, drain=nc.scalar.copy)
                proj_chunk(mc)
            # bias reductions deferred here: they only gate the ebc build
            # (~30us in), so keep them out of the early DVE feed window
            for t in range(MT):
                nc.vector.scalar_tensor_tensor(
                    bjunk[:], B16[t][:], 1.0, bvb[:],
                    op0=ALU.mult, op1=ALU.mult,
                    accum_out=bias_col[:, t:t + 1])
            # E = exp(bias); transpose [m%128, mt] -> [mt, m%128], assemble a
            # single row, and broadcast across partitions for the P multiply.
            nc.scalar.activation(ecol[:], bias_col[:], AF.Exp)
            ept = ps0.tile([PT, PT], FP16, name="ept", tag="ept", bufs=1)
            nc.tensor.transpose(ept[0:MT, :], ecol[:, 0:MT], ident16[:])
            nc.vector.tensor_copy(erow_t[0:MT, :], ept[0:MT, :])
            nc.scalar.dma_start(
                erow1[0:1, :].rearrange("a (t l) -> a t l", t=MT),
                erow_t[0:MT, :])
            nc.gpsimd.partition_broadcast(ebc[:], erow1[0:1, :])
            # A: bf16 stash for o2 (scaled in place later) + fp16 for S.
            for g in range(MC):
                grp = agrps[g]
                for q in range(4):
                    t = 4 * g + q
                    nc.gpsimd.tensor_copy(Asc[t][:], grp[:, q * D:(q + 1) * D])
                    nc.scalar.copy(A16[t][:], grp[:, q * D:(q + 1) * D])
                    tr_tile(t, A16[t], at_all, drain=nc.vector.tensor_copy)

        # ------ phase 1: per query-tile S -> P -> P^T (XBAR) -> o1 ---------
        # Software-pipelined: A^T transposes (4-tile groups) lead the S
        # stream by one group; o1(nt) trails S(nt) by O1_LAG so the exp/
        # rowsum/XBAR-transpose latency hides under the next S tiles.
        O1_LAG = 3
        p_pool = big.enter_context(tc.tile_pool(name=f"pp{sfx}", bufs=1))
        Pt = [p_pool.tile([PT, N2], BF16, name=f"p{t}", tag=f"p{t}") for t in range(NT)]
        with ExitStack() as ph12:
            ps1 = ph12.enter_context(tc.tile_pool(name=f"ps1{sfx}", bufs=1, space="PSUM"))
            smp = ph12.enter_context(tc.tile_pool(name=f"smp{sfx}", bufs=1))
            outp = ph12.enter_context(tc.tile_pool(name=f"outp{sfx}", bufs=1))

            def stage_s(nt):
                spsums = [ps1.tile([PT, CHUNK], F32, name="s", tag="s", bufs=4)
                          for _ in range(MC)]
                # mc-outer: each 512-key chunk finishes in 4 consecutive
                # MMs (Ldweights hide under 512-row matmuls), so its exp can
                # fire immediately and chunk 0 only needs projection chunk 0
                for mc in range(MC):
                    for k in range(KT):
                        nc.tensor.matmul(
                            spsums[mc][:],
                            at_all[:, k * N1 + nt * PT:k * N1 + (nt + 1) * PT],
                            CT[k][:, mc * CHUNK:(mc + 1) * CHUNK],
                            start=(k == 0), stop=(k == KT - 1),
                        )
                sums = []
                for mc in range(MC):
                    tch = smp.tile([PT, CHUNK], BF16, name="texp", tag="texp", bufs=4)
                    nc.scalar.activation(
                        tch[:], spsums[mc][:], AF.Exp,
                        bias=nshift[:], scale=1.0,
                    )
                    sm = smp.tile([PT, 1], F32, name="sum", tag="sum", bufs=8)
                    # P = T * E[m] (bias fold); accum gives the weighted rowsum
                    nc.vector.scalar_tensor_tensor(
                        Pt[nt][:, mc * CHUNK:(mc + 1) * CHUNK],
                        tch[:], 1.0,
                        ebc[:, mc * CHUNK:(mc + 1) * CHUNK],
                        op0=ALU.mult, op1=ALU.mult, accum_out=sm[:],
                    )
                    sums.append(sm)
                s01 = smp.tile([PT, 1], F32, name="s01", tag="s01", bufs=2)
                nc.vector.tensor_add(s01[:], sums[0][:], sums[1][:])
                s23 = smp.tile([PT, 1], F32, name="s23", tag="s23", bufs=2)
                nc.vector.tensor_add(s23[:], sums[2][:], sums[3][:])
                stot = smp.tile([PT, 1], F32, name="stot", tag="stot", bufs=2)
                nc.vector.tensor_add(stot[:], s01[:], s23[:])
                nc.vector.reciprocal(recip_all[:, nt:nt + 1], stot[:])
                # scale the stashed bf16 A tile in place by 1/rowsum
                nc.gpsimd.tensor_scalar_mul(Asc[nt][:], Asc[nt][:], recip_all[:, nt:nt + 1])
                # Guard: a tiny regular DMA reading one row of Pt[nt] on the
                # same HWDGE queue as the XBAR below. Its (reliable) sem
                # waits cover all four stt chunk writes, and in-order queue
                # dispatch then guarantees the XBAR reads finished data --
                # the XBAR ucode's own input waits proved untrustworthy.
                nc.sync.dma_start(gscr[0:1, :], Pt[nt][0:1, :])
                ptall = ptp.tile([PT, N2], BF16, name="ptx", tag="ptx", bufs=O1_LAG + 1)
                nc.sync.dma_start_transpose(
                    ptall[:].rearrange("p (b l) -> p b l", b=MT), Pt[nt][:])
                return ptall

            def stage_o1(nt, ptall):
                o1ps = ps1.tile([PT, D], F32, name="o1", tag="o1", bufs=2)
                for mt in range(MT):
                    nc.tensor.matmul(o1ps[:], ptall[:, mt * PT:(mt + 1) * PT],
                                     Bbf[mt][:],
                                     start=(mt == 0), stop=(mt == MT - 1))
                o1sb = outp.tile([PT, D], F32, name="o1sb", tag="o1sb", bufs=2)
                nc.scalar.mul(o1sb[:], o1ps[:], recip_all[:, nt:nt + 1])
                nc.sync.dma_start(o1_d[nt * PT:(nt + 1) * PT, :], o1sb[:])

            H = D // 2

            def stage_o2(mt):
                o2ps = ps1.tile([PT, D], F32, name="o2", tag="o2", bufs=2)
                o2sb = outp.tile([PT, D], F32, name="o2sb", tag="o2sb", bufs=2)
                for h in range(2):
                    for nt in range(NT):
                        nc.tensor.matmul(
                            o2ps[:, h * H:(h + 1) * H],
                            Pt[nt][:, mt * PT:(mt + 1) * PT],
                            Asc[nt][:, h * H:(h + 1) * H],
                            start=(nt == 0), stop=(nt == NT - 1))
                    cp2(mt + h, o2sb[:, h * H:(h + 1) * H], o2ps[:, h * H:(h + 1) * H])
                    if mt >= MT - 2:
                        nc.sync.dma_start(
                            o2_d[mt * PT:(mt + 1) * PT, h * H:(h + 1) * H],
                            o2sb[:, h * H:(h + 1) * H])
                if mt < MT - 2:
                    nc.sync.dma_start(o2_d[mt * PT:(mt + 1) * PT, :], o2sb[:])

            pts = {}
            for nt in range(NT):
                pts[nt] = stage_s(nt)
                if nt - O1_LAG >= 0:
                    stage_o1(nt - O1_LAG, pts.pop(nt - O1_LAG))
            # epilogue: pair each remaining o1 with an o2 tile so the PE has
            # ready work while the last XBAR transposes land
            mt_next = 0
            for nt in range(NT - O1_LAG, NT):
                stage_o2(mt_next); mt_next += 1
                stage_o1(nt, pts.pop(nt))
            for mt in range(mt_next, MT):
                stage_o2(mt)
        big.close()
    nc.compile()
    return nc


_state = {}


def _get_nc(nreps=1):
    key = f"nc{nreps}"
    if key not in _state:
        _state[key] = _build(nreps)
    return _state[key]


def _in_maps(input1, input2, W_w, W_b):
    return [
        {
            "a": np.ascontiguousarray(input1[bb], dtype=np.float32),
            "bm": np.ascontiguousarray(input2[bb], dtype=np.float32),
            "w": np.ascontiguousarray(W_w, dtype=np.float32),
            "bvec": np.ascontiguousarray(W_b, dtype=np.float32),
            "ones": np.ones((1, PT), dtype=np.float32),
            "eye": np.eye(PT, dtype=np.float32),
            "eye16": np.eye(PT, dtype=np.float16),
        }
        for bb in range(B)
    ]


def kernel(input1, input2, W_w, W_b):
    res = run_bass_kernel_spmd(
        _get_nc(), _in_maps(input1, input2, W_w, W_b), core_ids=list(range(B))
    )
    o1 = np.stack([r["o1"] for r in res.results])
    o2 = np.stack([r["o2"] for r in res.results])
    return o1, o2


def _pjrt_fn(nc, in_maps, donate=False):
    """Build a single-call jitted runner for `nc` (copy of run_bass_via_pjrt
    multi-core path, without donation so device inputs can be reused)."""
    import jax
    import numpy as np_
    from jax.sharding import Mesh, NamedSharding, PartitionSpec
    from jax.experimental.shard_map import shard_map

    from concourse import mybir as _mybir
    from concourse.bass2jax import (
        _bass_exec_p,
        install_neuronx_cc_hook,
        partition_id_tensor,
    )

    install_neuronx_cc_hook()
    partition_name = nc.partition_id_tensor.name if nc.partition_id_tensor else None

    in_names, out_names, out_avals, zero_outs = [], [], [], []
    for alloc in nc.m.functions[0].allocations:
        if not isinstance(alloc, _mybir.MemoryLocationSet):
            continue
        name = alloc.memorylocations[0].name
        if alloc.kind == "ExternalInput":
            if name != partition_name:
                in_names.append(name)
        elif alloc.kind == "ExternalOutput":
            out_names.append(name)
            shape = tuple(alloc.tensor_shape)
            dtype = _mybir.dt.np(alloc.dtype)
            out_avals.append(jax.core.ShapedArray(shape, dtype))
            zero_outs.append(np_.zeros(shape, dtype))

    all_in = list(in_names) + list(out_names)
    if partition_name is not None:
        all_in.append(partition_name)

    def _body(*args):
        operands = list(args)
        if partition_name is not None:
            operands.append(partition_id_tensor())
        outs = _bass_exec_p.bind(
            *operands,
            out_avals=tuple(out_avals),
            in_names=tuple(all_in),
            out_names=tuple(out_names),
            lowering_input_output_aliases=(),
            sim_require_finite=True,
            sim_require_nnan=True,
            nc=nc,
        )
        return tuple(outs)

    devices = jax.devices()[:B]
    mesh = Mesh(np_.asarray(devices), ("core",))
    nargs = len(in_names) + len(out_names)
    sh = NamedSharding(mesh, PartitionSpec("core"))
    fn = jax.jit(
        shard_map(
            _body, mesh=mesh,
            in_specs=(PartitionSpec("core"),) * nargs,
            out_specs=(PartitionSpec("core"),) * len(out_names),
            check_rep=False,
        ),
        **({"donate_argnums": tuple(range(len(in_names), nargs))} if donate else {}),
    )
    args = [
        jax.device_put(np_.concatenate([m[n] for m in in_maps], axis=0), sh)
        for n in in_names
    ] + [
        jax.device_put(np_.concatenate([z] * B, axis=0), sh) for z in zero_outs
    ]
    return fn, args, out_names, out_avals


def _time_fn(fn, args, calls=30, reps=4):
    """Pipelined timing: issue `calls` executions, block once at the end.
    Returns list of per-call ns (one value per rep)."""
    import time

    import jax

    r = fn(*args)
    jax.block_until_ready(r)
    out = []
    for _ in range(reps):
        t0 = time.perf_counter()
        for _ in range(calls):
            r = fn(*args)
        jax.block_until_ready(r)
        out.append((time.perf_counter() - t0) / calls * 1e9)
    return out


def bench_hw(input1, input2, W_w, W_b, calls=40):
    """HW body time via 2-rep minus 1-rep NEFF wall times (dispatch cancels).
    Returns (body_ns, t1_list_p, t2_list_p)."""
    in_maps = _in_maps(input1, input2, W_w, W_b)
    fn1, args1, _, _ = _pjrt_fn(_get_nc(1), in_maps)
    fn2, args2, _, _ = _pjrt_fn(_get_nc(2), in_maps)
    t1 = _time_fn(fn1, args1, calls)
    t2 = _time_fn(fn2, args2, calls)
    import numpy as np_
    p = lambda ts, q: float(np_.percentile(ts, q))
    body = p(t2, 10) - p(t1, 10)
    return body, (p(t1,10), p(t1,50)), (p(t2,10), p(t2,50))
